# revision 3
# baseline (speedup 1.0000x reference)
"""Causal self-attention with RoPE for trn2, 8-core SPMD — fused v2.

Sharding: core i handles batch b = i//2 and heads [8*(i%2), 8*(i%2)+8).
Each core computes a partial output [T, C]; host sums core pairs + bo.

v2 vs baseline: everything stays in SBUF (no q/k/v/y DRAM round-trips),
matmul operands are bf16 (host-cast), attention diag blocks are narrowed
to the causal width, and projections/attention stream per head.

Per-core layout (partition dim first):
  xT_sb [128, 16ct, 2048t] bf16   resident until last head's q/k
  v_sb  [128t, 16tt, 8h, 128d] bf16  resident
  y_sb  [128d, 8h, 2048t] bf16       resident
  q/k per head [128d, 2048t] bf16, double-buffered
P2 per head: St[tk,tq] narrowed to causal width; E=exp on ACT (bf16);
  triangular 128x128 mask on DVE for diag sub-blocks; denom via
  ones-matmul + AV matmul, two blocks behind St; normalize on eviction.
P3: out[t,c] = sum_h y_h[:,t-tile].T @ Wo_h rows, straight from SBUF.
"""
from contextlib import ExitStack

import numpy as np

import concourse.bacc as bacc
import concourse.tile as tile
from concourse import mybir

F32 = mybir.dt.float32
BF16 = mybir.dt.bfloat16
AF = mybir.ActivationFunctionType
ALU = mybir.AluOpType

N_HEAD = 16
HEAD_DIM = 128
ROPE_BASE = 10000.0


def build_core_kernel(T=2048, C=2048, HL=8, reps=1, pool_mode="queue",
                      unroll=1):
    D = HEAD_DIM
    CL = HL * D            # local q/k/v channels (1024)
    NCT = C // 128         # c-tiles (16)
    NQ = T // 512          # 512-wide q slices (4)
    NT = T // 128          # 128-wide t tiles (16)
    NG = CL // 512         # v column groups (2)
    NCQ = C // 512         # out-proj column groups (4)
    scale = 1.0 / float(np.sqrt(D))

    nc = bacc.Bacc("TRN2", target_bir_lowering=False, debug=False)

    xT_d = nc.dram_tensor("xT", [NCT, 128, T], BF16, kind="ExternalInput")
    wq_d = nc.dram_tensor("wq", [HL, 128, NCT, 128], BF16, kind="ExternalInput")
    wk_d = nc.dram_tensor("wk", [HL, 128, NCT, 128], BF16, kind="ExternalInput")
    wv_d = nc.dram_tensor("wv", [NG, 128, NCT, 512], BF16, kind="ExternalInput")
    wo_d = nc.dram_tensor("wo", [128, HL, NCQ, 512], BF16, kind="ExternalInput")
    bq_d = nc.dram_tensor("bq", [CL], F32, kind="ExternalInput")
    bk_d = nc.dram_tensor("bk", [CL], F32, kind="ExternalInput")
    bqs_d = nc.dram_tensor("bqs", [CL], F32, kind="ExternalInput")
    bks_d = nc.dram_tensor("bks", [CL], F32, kind="ExternalInput")
    bv_d = nc.dram_tensor("bv", [CL], F32, kind="ExternalInput")
    cos_d = nc.dram_tensor("cos2", [128, T], F32, kind="ExternalInput")
    sin_d = nc.dram_tensor("sin2s", [128, T], F32, kind="ExternalInput")
    mask_d = nc.dram_tensor("masktri", [128, 128], BF16, kind="ExternalInput")
    ones_d = nc.dram_tensor("ones", [128, 128], BF16, kind="ExternalInput")
    out_d = nc.dram_tensor("out_p", [T, C], F32, kind="ExternalOutput")

    with tile.TileContext(nc, pool_alloc_mode=pool_mode) as tc, ExitStack() as top:
        psA = top.enter_context(tc.tile_pool(name="psA", bufs=4, space="PSUM"))
        psY = top.enter_context(tc.tile_pool(name="psY", bufs=2, space="PSUM"))
        psD = top.enter_context(tc.tile_pool(name="psD", bufs=2, space="PSUM"))

        const = top.enter_context(tc.tile_pool(name="const", bufs=1))
        ones_sb = const.tile([128, 128], BF16)
        mask_sb = const.tile([128, 128], BF16)
        cos_sb = const.tile([128, T], F32)
        sin_sb = const.tile([128, T], F32)
        bq_sb = const.tile([128, HL], F32)
        bk_sb = const.tile([128, HL], F32)
        bqs_sb = const.tile([128, HL], F32)
        bks_sb = const.tile([128, HL], F32)
        bv_sb = const.tile([128, CL], F32)
        nc.sync.dma_start(out=ones_sb, in_=ones_d[:, :])
        nc.sync.dma_start(out=mask_sb, in_=mask_d[:, :])
        nc.sync.dma_start(out=cos_sb, in_=cos_d[:, :])
        nc.sync.dma_start(out=sin_sb, in_=sin_d[:, :])
        nc.sync.dma_start(out=bq_sb, in_=bq_d.rearrange("(h p) -> p h", p=128))
        nc.sync.dma_start(out=bk_sb, in_=bk_d.rearrange("(h p) -> p h", p=128))
        nc.sync.dma_start(out=bqs_sb, in_=bqs_d.rearrange("(h p) -> p h", p=128))
        nc.sync.dma_start(out=bks_sb, in_=bks_d.rearrange("(h p) -> p h", p=128))
        nc.sync.dma_start(out=bv_sb, in_=bv_d[:].partition_broadcast(128))

        rep_ctx = tc.For_i(0, reps, 1) if reps > 1 else None
        if rep_ctx is not None:
            top.enter_context(rep_ctx)

        for _u in range(unroll):
          with ExitStack() as body:
            vy = body.enter_context(tc.tile_pool(name="vy", bufs=1))
            v_sb = vy.tile([128, NT, HL, 128], BF16)
            y_sb = vy.tile([128, HL, T], BF16)

            with ExitStack() as s1:
                xp = s1.enter_context(tc.tile_pool(name="xp", bufs=1))
                xT_sb = xp.tile([128, NCT, T], BF16)

                # ---- v for all heads ----
                # (g=0 weight tiles queued before the 8MB xT load so the
                # first chain isn't stuck behind it)
                with ExitStack() as pv:
                    wvp = pv.enter_context(tc.tile_pool(name="wvp", bufs=1))
                    wv0 = wvp.tile([128, NCT, 512], BF16, tag="wv")
                    for ct in range(NCT):
                        nc.sync.dma_start(
                            out=wv0[:, ct, :], in_=wv_d[0, :, ct, :]
                        )
                    for ct in range(NCT):
                        nc.sync.dma_start(out=xT_sb[:, ct, :], in_=xT_d[ct])
                    for g in range(NG):
                        gs = slice(g * 512, (g + 1) * 512)
                        if g == 0:
                            wv_sb = wv0
                        else:
                            wv_sb = wvp.tile([128, NCT, 512], BF16, tag="wv")
                            for ct in range(NCT):
                                nc.sync.dma_start(
                                    out=wv_sb[:, ct, :], in_=wv_d[g, :, ct, :]
                                )
                        for tt in range(NT):
                            xl = xT_sb[:, :, tt * 128 : (tt + 1) * 128]
                            ps = psA.tile([128, 512], F32, tag="mm")
                            for ct in range(NCT):
                                nc.tensor.matmul(
                                    ps[:],
                                    xl[:, ct, :],
                                    wv_sb[:, ct, :],
                                    start=(ct == 0),
                                    stop=(ct == NCT - 1),
                                )
                            nc.vector.tensor_tensor(
                                v_sb[:, tt, g * 4 : (g + 1) * 4, :],
                                ps[:], bv_sb[:, gs], op=ALU.add,
                            )

                # ---- per head: projections+rope, then attention ----
                qkp = s1.enter_context(tc.tile_pool(name="qkp", bufs=2))
                wqk = s1.enter_context(tc.tile_pool(name="wqk", bufs=2))
                ev = s1.enter_context(tc.tile_pool(name="ev", bufs=2))
                ep = s1.enter_context(tc.tile_pool(name="ep", bufs=6))
                rp = s1.enter_context(tc.tile_pool(name="rp", bufs=2))

                def proj_slice(w_sb, b_sb, bs_sb, o_sb, h, s):
                    ts = slice(s * 512, (s + 1) * 512)
                    ps = psA.tile([128, 512], F32, tag="mm")
                    for ct in range(NCT):
                        nc.tensor.matmul(
                            ps[:],
                            w_sb[:, ct, :],
                            xT_sb[:, ct, ts],
                            start=(ct == 0),
                            stop=(ct == NCT - 1),
                        )
                    # rope straight off PSUM, all on DVE:
                    #   swp = shuffle(ps); p1 = (ps+b)*cos;
                    #   swp = (swp+b_shuf)*sin; out = p1 + swp
                    # pair partner is +-16 within each 32-partition quadrant
                    # (host layout: 16 x1 rows then 16 x2 rows per quadrant).
                    swp = ev.tile([128, 512], F32, tag="swp")
                    nc.vector.stream_shuffle(
                        swp[:], ps[:], mask=[(i + 16) % 32 for i in range(32)]
                    )
                    p1 = ev.tile([128, 512], F32, tag="raw")
                    nc.vector.scalar_tensor_tensor(
                        out=p1[:], in0=ps[:], scalar=b_sb[:, h : h + 1],
                        in1=cos_sb[:, ts], op0=ALU.add, op1=ALU.mult,
                    )
                    nc.vector.scalar_tensor_tensor(
                        out=swp[:], in0=swp[:], scalar=bs_sb[:, h : h + 1],
                        in1=sin_sb[:, ts], op0=ALU.add, op1=ALU.mult,
                    )
                    nc.vector.tensor_tensor(
                        o_sb[:, ts], p1[:], swp[:], op=ALU.add
                    )

                def attn_j(q_sb, k_sb, h, j):
                    nblk = 4 * (j + 1)
                    psd = psD.tile([128, 512], F32, name="psd", tag="psd")
                    psy = psY.tile([128, 512], F32, name="psy", tag="psy")
                    ets = []
                    LOOK = 2
                    for b in range(nblk + LOOK):
                        if b < nblk:
                            mi = b - 4 * j
                            off = mi * 128 if mi > 0 else 0
                            pss = psA.tile([128, 512], F32, tag="mm")
                            nc.tensor.matmul(
                                pss[:, off:],
                                k_sb[:, b * 128 : (b + 1) * 128],
                                q_sb[:, j * 512 + off : (j + 1) * 512],
                                start=True, stop=True,
                            )
                            et = ep.tile([128, 512], BF16, tag="et")
                            nc.scalar.activation(
                                out=et[:, off:], in_=pss[:, off:],
                                func=AF.Exp, scale=scale,
                            )
                            if mi >= 0:
                                nc.vector.tensor_tensor(
                                    et[:, off : off + 128],
                                    et[:, off : off + 128],
                                    mask_sb[:], op=ALU.mult,
                                )
                            ets.append((et, off))
                        d = b - LOOK
                        if 0 <= d < nblk:
                            edt, eoff = ets[d]
                            nc.tensor.matmul(
                                psd[:, eoff:], ones_sb[:], edt[:, eoff:],
                                start=(d == 0), stop=(d == nblk - 1),
                            )
                            nc.tensor.matmul(
                                psy[:, eoff:], v_sb[:, d, h, :], edt[:, eoff:],
                                start=(d == 0), stop=(d == nblk - 1),
                            )
                    recb = rp.tile([128, 512], F32, tag="recb")
                    nc.vector.reciprocal(out=recb[:], in_=psd[:])
                    nc.vector.tensor_tensor(
                        y_sb[:, h, j * 512 : (j + 1) * 512],
                        psy[:], recb[:], op=ALU.mult,
                    )

                def alloc_w(h):
                    wk_t = wqk.tile([128, NCT, 128], BF16, name="wk_t",
                                    tag="wks")
                    wq_t = wqk.tile([128, NCT, 128], BF16, name="wq_t",
                                    tag="wqs")
                    nc.sync.dma_start(out=wk_t, in_=wk_d[h])
                    nc.sync.dma_start(out=wq_t, in_=wq_d[h])
                    return wk_t, wq_t

                nxt_w = alloc_w(0)
                for h in range(HL):
                    q_sb = qkp.tile([128, T], BF16, tag="qh")
                    k_sb = qkp.tile([128, T], BF16, tag="kh")
                    wk_sb, wq_sb = nxt_w
                    if h + 1 < HL:
                        # prefetch next head's weights into the other
                        # double-buffer slot while this head computes
                        nxt_w = alloc_w(h + 1)
                    for s in range(NQ):
                        proj_slice(wk_sb, bk_sb, bks_sb, k_sb, h, s)
                    proj_slice(wq_sb, bq_sb, bqs_sb, q_sb, h, 0)
                    proj_slice(wq_sb, bq_sb, bqs_sb, q_sb, h, 1)
                    attn_j(q_sb, k_sb, h, 0)
                    proj_slice(wq_sb, bq_sb, bqs_sb, q_sb, h, 2)
                    attn_j(q_sb, k_sb, h, 1)
                    proj_slice(wq_sb, bq_sb, bqs_sb, q_sb, h, 3)
                    attn_j(q_sb, k_sb, h, 2)
                    attn_j(q_sb, k_sb, h, 3)

            # ---- output projection (wo reuses xT address space) ----
            with ExitStack() as p3:
                wop = p3.enter_context(tc.tile_pool(name="wop", bufs=1))
                op = p3.enter_context(tc.tile_pool(name="op", bufs=3))
                wo_sb = wop.tile([128, HL, NCQ, 512], BF16)
                nc.sync.dma_start(out=wo_sb, in_=wo_d[:, :, :, :])
                for tt in range(NT):
                    tsl = slice(tt * 128, (tt + 1) * 128)
                    ot = op.tile([128, C], F32)
                    for cq in range(NCQ):
                        ps = psA.tile([128, 512], F32, tag="mm")
                        for h in range(HL):
                            nc.tensor.matmul(
                                ps[:],
                                y_sb[:, h, tsl],
                                wo_sb[:, h, cq, :],
                                start=(h == 0),
                                stop=(h == HL - 1),
                            )
                        nc.scalar.copy(
                            out=ot[:, cq * 512 : (cq + 1) * 512], in_=ps[:]
                        )
                    nc.sync.dma_start(out=out_d[tsl, :], in_=ot[:])

    nc.finalize()
    return nc


def _col_perm(CL):
    """Per-head quadrant-local de-interleave: each 32-partition quadrant
    holds 16 x1 (even) rows then 16 x2 (odd) rows, so the rope pair swap
    is a +-16 rotation within a quadrant (DVE stream_shuffle)."""
    perm = []
    for h in range(CL // 128):
        base = h * 128
        for qd in range(4):
            perm += [base + 2 * (16 * qd + i) for i in range(16)]
            perm += [base + 2 * (16 * qd + i) + 1 for i in range(16)]
    return np.array(perm)


def _bf16(a):
    import ml_dtypes
    return np.ascontiguousarray(a.astype(ml_dtypes.bfloat16))


def host_prepare(x, Wq, bq, Wk, bk, Wv, bv, Wo, bo, T=None):
    """Build the 8 per-core input maps. x: [B, T, C] fp32."""
    B, Tfull, C = x.shape
    if T is None:
        T = Tfull
    D = HEAD_DIM
    NCT = C // 128
    perm = _col_perm(C)
    Wq_p = np.ascontiguousarray(Wq[:, perm])
    Wk_p = np.ascontiguousarray(Wk[:, perm])
    bq_p = np.ascontiguousarray(bq[perm])
    bk_p = np.ascontiguousarray(bk[perm])

    inv = (1.0 / (ROPE_BASE ** (np.arange(0, D, 2, dtype=np.float32) / D))).astype(
        np.float32
    )
    pos = np.arange(T, dtype=np.float32)
    th = pos[None, :] * inv[:, None]          # [64, T]
    cos1 = np.cos(th).astype(np.float32)
    sin1 = np.sin(th).astype(np.float32)
    # rows follow the quadrant-local x1/x2 layout of _col_perm:
    # row r: quadrant r//32, x1 if r%32<16; freq idx = 16*(r//32) + r%16
    r = np.arange(128)
    fi = (r // 32) * 16 + (r % 16)
    sign = np.where((r % 32) < 16, -1.0, 1.0).astype(np.float32)
    cos2 = cos1[fi]
    # pre-swapped sign layout: rot = raw*cos2 + shuffle(raw)*sin2s
    sin2s = sin1[fi] * sign[:, None]

    p = np.arange(128)[:, None]
    f = np.arange(128)[None, :]
    masktri = (p <= f).astype(np.float32)      # E[tk, tq] allowed tq >= tk
    # partition permutation performed by the rope stream_shuffle
    rr = np.arange(128)
    shufperm = (rr // 32) * 32 + ((rr % 32) + 16) % 32

    def tile_qk(W):  # [C, 1024] -> [8, 128, NCT, 128]
        return W.reshape(NCT, 128, 8, 128).transpose(2, 1, 0, 3)

    def tile_v(W):  # [C, 1024] -> [2, 128, NCT, 512]
        return W.reshape(NCT, 128, 2, 512).transpose(2, 1, 0, 3)

    def tile_o(W):  # [1024, C] -> [128, 8, C//512, 512]
        return W.reshape(8, 128, C // 512, 512).transpose(1, 0, 2, 3)

    in_maps = []
    for core in range(8):
        b, half = core // 2, core % 2
        cl = slice(half * 1024, (half + 1) * 1024)
        xT = x[b, :T].T.reshape(NCT, 128, T)
        in_maps.append(
            {
                "xT": _bf16(xT),
                "wq": _bf16(tile_qk(Wq_p[:, cl])),
                "wk": _bf16(tile_qk(Wk_p[:, cl])),
                "wv": _bf16(tile_v(Wv[:, cl])),
                "wo": _bf16(tile_o(Wo[cl.start : cl.stop, :])),
                "bq": np.ascontiguousarray(bq_p[cl]),
                "bk": np.ascontiguousarray(bk_p[cl]),
                "bqs": np.ascontiguousarray(
                    bq_p[cl].reshape(8, 128)[:, shufperm].reshape(-1)
                ),
                "bks": np.ascontiguousarray(
                    bk_p[cl].reshape(8, 128)[:, shufperm].reshape(-1)
                ),
                "bv": np.ascontiguousarray(bv[cl]),
                "cos2": cos2,
                "sin2s": sin2s,
                "masktri": _bf16(masktri),
                "ones": _bf16(np.ones((128, 128), dtype=np.float32)),
            }
        )
    return in_maps


def assemble(results, bo, B, T, C):
    out = np.empty((B, T, C), dtype=np.float32)
    for b in range(B):
        out[b] = results[2 * b]["out_p"] + results[2 * b + 1]["out_p"] + bo[None, :]
    return out


# ---------------------------------------------------------------------------
# SPMD execution via PJRT/axon (compiles once per process, reusable)
# ---------------------------------------------------------------------------
import jax
from jax.sharding import Mesh, PartitionSpec
from jax.experimental.shard_map import shard_map

from concourse.bass2jax import (
    _bass_exec_p,
    install_neuronx_cc_hook,
    partition_id_tensor,
)


class _SpmdRunner:
    def __init__(self, nc, n_cores):
        install_neuronx_cc_hook()
        self.nc = nc
        self.n_cores = n_cores
        partition_name = (
            nc.partition_id_tensor.name if nc.partition_id_tensor else None
        )
        in_names, out_names, out_avals, zero_outs = [], [], [], []
        for alloc in nc.m.functions[0].allocations:
            if not isinstance(alloc, mybir.MemoryLocationSet):
                continue
            name = alloc.memorylocations[0].name
            if alloc.kind == "ExternalInput":
                if name != partition_name:
                    in_names.append(name)
            elif alloc.kind == "ExternalOutput":
                shape = tuple(alloc.tensor_shape)
                dtype = mybir.dt.np(alloc.dtype)
                out_names.append(name)
                out_avals.append(jax.core.ShapedArray(shape, dtype))
                zero_outs.append(np.zeros(shape, dtype))
        n_params = len(in_names)
        all_in_names = list(in_names) + list(out_names)
        if partition_name is not None:
            all_in_names.append(partition_name)
        self.in_names, self.out_names = in_names, out_names
        self.out_avals, self.zero_outs = out_avals, zero_outs

        def _body(*args):
            operands = list(args)
            if partition_name is not None:
                operands.append(partition_id_tensor())
            outs = _bass_exec_p.bind(
                *operands,
                out_avals=tuple(out_avals),
                in_names=tuple(all_in_names),
                out_names=tuple(out_names),
                lowering_input_output_aliases=(),
                sim_require_finite=True,
                sim_require_nnan=True,
                nc=nc,
            )
            return tuple(outs)

        devices = jax.devices()[:n_cores]
        assert len(devices) == n_cores, (
            f"need {n_cores} neuron cores, found {len(jax.devices())}"
        )
        mesh = Mesh(np.asarray(devices), ("core",))
        n_outs = len(out_avals)
        self.sharding = jax.sharding.NamedSharding(mesh, PartitionSpec("core"))
        self.fn = jax.jit(
            shard_map(
                _body,
                mesh=mesh,
                in_specs=(PartitionSpec("core"),) * (n_params + n_outs),
                out_specs=(PartitionSpec("core"),) * n_outs,
                check_rep=False,
            ),
            keep_unused=True,
        )

    def run(self, in_maps):
        n = self.n_cores
        concat_in = [
            np.concatenate(
                [np.asarray(in_maps[c][name]) for c in range(n)], axis=0
            )
            for name in self.in_names
        ]
        concat_zero = [
            np.zeros((n * z.shape[0], *z.shape[1:]), z.dtype)
            for z in self.zero_outs
        ]
        out_arrs = self.fn(*concat_in, *concat_zero)
        jax.block_until_ready(out_arrs)
        return [
            {
                name: np.asarray(out_arrs[i]).reshape(
                    n, *self.out_avals[i].shape
                )[c]
                for i, name in enumerate(self.out_names)
            }
            for c in range(8)
        ]


_RUNNER_CACHE = {}


def _get_runner(reps=1):
    key = reps
    if key not in _RUNNER_CACHE:
        nc = build_core_kernel(T=2048, C=2048, HL=8, reps=reps, pool_mode="queue")
        _RUNNER_CACHE[key] = _SpmdRunner(nc, 8)
    return _RUNNER_CACHE[key]


def kernel(x, Wq, bq, Wk, bk, Wv, bv, Wo, bo, _reps=1):
    """Causal self-attention with RoPE. Full inputs in, full output out."""
    x = np.ascontiguousarray(np.asarray(x, dtype=np.float32))
    B, T, C = x.shape
    in_maps = host_prepare(
        np.asarray(x), np.asarray(Wq), np.asarray(bq), np.asarray(Wk),
        np.asarray(bk), np.asarray(Wv), np.asarray(bv), np.asarray(Wo),
        np.asarray(bo),
    )
    runner = _get_runner(_reps)
    results = runner.run(in_maps)
    return assemble(results, np.asarray(bo, dtype=np.float32), B, T, C)
